# revision 1
# baseline (speedup 1.0000x reference)
"""Channel-attention module (CAM) forward for Trainium2.

Computes, per batch b:
    f1 = x[b].reshape(C, H*W)                      # [512, 4096]
    S  = f1 @ f1.T                                 # [512, 512]
    G  = softmax(S_max - S, axis=-1)               # == exp(S_min - S) / rowsum
    fc = G @ f1
    y[b] = beta * fc + x[b]

Sharding: data-parallel over batch B=16 across 8 NeuronCores (2 batches/core),
no cross-core communication. Matmuls run in bf16 on the PE array with fp32
PSUM accumulation; softmax statistics and the final residual combine are fp32.

Per-core dataflow (per batch), tuned against the Tile cost-model timeline:
  - x loads as 8 big [128, 2048] fp32 SWDGE tiles (resident: they feed the
    bf16 casts, the fc rhs casts, and the final fp32 residual add).
  - f1^T bf16 via ACT fp32->bf16 casts + 8 big DMA xbar transposes into two
    half-K buffers. All plain DMAs ride SWDGE (gpsimd) and only these 8
    transposes use HWDGE: the scheduler hard-serializes xbar-mode transitions
    between DmaTranspose and any DMACopy, so the DMA stream is arranged as
    [loads b][xposes b][loads b+1][xposes b+1][stores b][stores b+1] with
    only two mode transitions per batch.
  - S runs kh-outer / m-mid / k-inner: kh=1 transposes overlap kh=0 matmuls,
    and each row-block's softmax fires as soon as its kh=1 block finishes.
  - Softmax: DVE row-min, ACT exp (bias=rowmin, scale=-1) with fp32 row-sum
    accum_out. beta/Z is computed as beta*exp(-ln Z) entirely on ACT right
    after the exp (same-engine program order => no sequencer stalls), and is
    applied in the fused epilogue, so G stays unnormalized.
  - G^T on the PE (16 transpose-mode matmuls into freed S psum banks + 4 ACT
    psum->sbuf copies) to keep it off the DMA stream.
  - fc runs j-outer (one DVE bf16 rhs cast per (kt, j), reused by 4
    m-blocks), accumulating in 4 PSUM banks; the epilogue is one fused DVE
    scalar_tensor_tensor: y = (beta/Z)[c]*fc_raw + x, stored via SWDGE.
"""

import numpy as np

B, C, HW = 16, 512, 4096
NCORES = 8
BL = B // NCORES  # batches per core
P = 128
CT = C // P       # 4 c-tiles of 128 channels
NCH = 8           # n-chunks of 512
F = 512           # n-chunk size (psum free dim)
HALF = HW // 2    # 2048, the load/cast/transpose granularity
KH = 16           # 128-wide k-tiles per half

_CACHE = {}


def _build():
    import concourse.bass as bass  # noqa: F401
    import concourse.mybir as mybir
    import concourse.tile as tile
    from concourse import bacc
    from concourse.masks import make_identity

    f32 = mybir.dt.float32
    bf16 = mybir.dt.bfloat16
    AF = mybir.ActivationFunctionType
    OP = mybir.AluOpType
    AX = mybir.AxisListType

    # Bacc (not plain Bass): its compile() runs generate_event_semaphores,
    # which splits excess sync waits onto EventSemaphore instructions —
    # required because TRN2 instructions (notably DMA_DIRECT2D_XPOSE) carry
    # at most one wait.
    nc = bacc.Bacc("TRN2", target_bir_lowering=False, debug=False)
    x_d = nc.dram_tensor("x", [BL, C, HW], f32, kind="ExternalInput")
    beta_d = nc.dram_tensor("beta", [1], f32, kind="ExternalInput")
    y_d = nc.dram_tensor("y", [BL, C, HW], f32, kind="ExternalOutput")

    with tile.TileContext(nc) as tc:
        with (
            tc.tile_pool(name="singles", bufs=1) as singles,
            tc.tile_pool(name="xf", bufs=16) as xf,          # [128,2048] f32
            tc.tile_pool(name="stage", bufs=3) as stage_p,   # [128,2048] bf16
            tc.tile_pool(name="f1t", bufs=2) as f1t_p,       # [128,16,512] bf16
            tc.tile_pool(name="gst", bufs=2) as gst_p,       # [128,4,512] bf16
            tc.tile_pool(name="soft", bufs=4) as soft_p,     # [128,1] f32
            tc.tile_pool(name="ebf", bufs=4) as ebf_p,       # [128,512] bf16
            # 4 bufs: all four e[m] are live at the S->fc boundary; with
            # fewer, exp[m+2] stalls on the PE transposes releasing a slot
            tc.tile_pool(name="rhs", bufs=4) as rhs_p,       # [128,512] bf16
            tc.tile_pool(name="outs", bufs=3) as out_p,      # [128,1024] f32
            tc.tile_pool(name="ps_s", bufs=4, space="PSUM") as ps_s,
            tc.tile_pool(name="ps_fc", bufs=4, space="PSUM") as ps_fc,
        ):
            beta_sb = singles.tile([P, 1], f32)
            nc.gpsimd.dma_start(out=beta_sb[:], in_=beta_d[:].to_broadcast([P, 1]))
            ident = singles.tile([P, P], bf16)
            make_identity(nc, ident[:])

            from bass_rust import add_dep_helper

            # The scheduler hard-serializes every xbar-mode transition in the
            # scheduled DMA order (DmaTranspose <-> any DMACopy, both SWDGE
            # and HWDGE). Left alone it interleaves loads and transposes,
            # turning the DMA system into a ping-pong serial chain. We pin a
            # coherent segment order instead:
            #   [loads h][xposes h] per half, then next batch, then stores.
            # One cross-engine dep per boundary suffices: each engine's
            # sequencer is in-order, loads/stores issue from Pool and
            # transposes from ACT.
            last_xpose = [None]  # most recent transpose instruction

            # x loads + bf16 cast (ACT) + big DMA xbar transpose (HWDGE),
            # emitted per half so the S kh=0 matmuls can start after ~half
            # the DMA prep
            def emit_loads_prep(b):
                xts = {}
                f1t = []
                for h in range(2):
                    fh = f1t_p.tile([P, KH, F], bf16, tag="f1t", name=f"f1t_{b}_{h}")
                    last_load = None
                    for ct in range(CT):
                        t = xf.tile([P, HALF], f32, tag="xf", name=f"x_{b}_{ct}_{h}")
                        ld = nc.gpsimd.dma_start(
                            out=t[:],
                            in_=x_d[b, ct * P : (ct + 1) * P, h * HALF : (h + 1) * HALF],
                        )
                        if ct == 0 and last_xpose[0] is not None:
                            add_dep_helper(
                                ld.ins,
                                last_xpose[0],
                                reason="xbar segment: loads after prior xposes",
                            )
                        last_load = ld.ins
                        xts[(ct, h)] = t
                    for ct in range(CT):
                        st = stage_p.tile(
                            [P, HALF], bf16, tag="stage", name=f"st_{b}_{ct}_{h}"
                        )
                        nc.scalar.copy(out=st[:], in_=xts[(ct, h)][:])
                        # MUST issue from ACT: SP-issued dma_start_transpose
                        # reproducibly faults the device (the known TC5 hang
                        # that moved hwdge transposes off SP upstream)
                        xp = nc.scalar.dma_start_transpose(
                            fh[:, :, ct * P : (ct + 1) * P], st[:]
                        )
                        if ct == 0:
                            add_dep_helper(
                                xp.ins,
                                last_load,
                                reason="xbar segment: xposes after this half's loads",
                            )
                        last_xpose[0] = xp.ins
                    f1t.append(fh)
                return xts, f1t

            xt_all = {}
            f1t_all = {}
            xt_all[0], f1t_all[0] = emit_loads_prep(0)
            first_store = {b: None for b in range(BL)}

            for b in range(BL):
                xt = xt_all[b]
                f1t = f1t_all[b]

                # ---- S = f1 @ f1^T : kh-outer / m-mid / k-inner ----
                s_ps = [
                    ps_s.tile([P, F], f32, tag="s", name=f"s_ps_{b}_{m}")
                    for m in range(CT)
                ]
                es, zs, brs = [], [], []
                for h in range(2):
                    for m in range(CT):
                        for kl in range(KH):
                            nc.tensor.matmul(
                                s_ps[m][:],
                                lhsT=f1t[h][:, kl, m * P : (m + 1) * P],
                                rhs=f1t[h][:, kl, :],
                                start=(h == 0 and kl == 0),
                                stop=(h == 1 and kl == KH - 1),
                            )
                        if h == 1:
                            # row stats of S: min (softmax shift) + rowsum of
                            # exp. G stays UNNORMALIZED (exp(S_min - S)); the
                            # beta/Z row scale is folded into the epilogue.
                            mn = soft_p.tile([P, 1], f32, tag="mn", name=f"mn{b}{m}")
                            nc.vector.tensor_reduce(
                                out=mn[:], in_=s_ps[m][:], axis=AX.X, op=OP.min
                            )
                            e = ebf_p.tile([P, F], bf16, tag="e", name=f"e{b}{m}")
                            z = soft_p.tile([P, 1], f32, tag="z", name=f"z{b}{m}")
                            nc.scalar.activation(
                                out=e[:],
                                in_=s_ps[m][:],
                                func=AF.Exp,
                                bias=mn[:],
                                scale=-1.0,
                                accum_out=z[:],
                            )
                            es.append(e)
                            zs.append(z)

                # beta/Z via beta*exp(-ln Z) on ACT (a DVE reciprocal would
                # block DVE.SEQ until S completes, starving fc). Emitted after
                # all exps so the in-order ACT stream reaches exp[m] without
                # detours.
                for m in range(CT):
                    lz = soft_p.tile([P, 1], f32, tag="lz", name=f"lz{b}{m}")
                    nc.scalar.activation(out=lz[:], in_=zs[m][:], func=AF.Ln)
                    br = soft_p.tile([P, 1], f32, tag="br", name=f"br{b}{m}")
                    nc.scalar.activation(
                        out=br[:], in_=lz[:], func=AF.Exp, scale=-1.0
                    )
                    nc.scalar.mul(out=br[:], in_=br[:], mul=beta_sb[:])
                    brs.append(br)

                # ---- G^T on the PE: 4 transpose-mode matmuls per row-block
                #      into a freed S psum bank, one ACT copy out per m ----
                # xp tiles reuse the S psum slots (freed once exp[m] has read
                # S[m]), keeping 4 banks available for the fc accumulators
                gst = gst_p.tile([P, CT, C], bf16, tag="gst", name=f"gst_{b}")
                for m in range(CT):
                    xp = ps_s.tile([P, CT, P], bf16, tag="s", name=f"xp_{b}_{m}")
                    for dt in range(CT):
                        nc.tensor.transpose(
                            xp[:, dt, :], es[m][:, dt * P : (dt + 1) * P], ident[:]
                        )
                    nc.scalar.copy(out=gst[:, :, m * P : (m + 1) * P], in_=xp[:])

                # next batch's loads + casts + transposes are emitted before
                # this batch's fc so its DMA transposes precede this batch's
                # stores in the scheduled DMA order (fewer xbar-mode stalls),
                # and so PE can roll into S(b+1) right after fc(b)
                if b + 1 < BL:
                    xt_all[b + 1], f1t_all[b + 1] = emit_loads_prep(b + 1)

                # ---- fc = G_raw @ f1 ; y = (beta/Z) * fc_raw + x ----
                for j in range(NCH):
                    h, jj = j // 4, j // 2
                    jo = (j % 4) * F  # offset within the half-tile
                    for kt in range(CT):
                        rt = rhs_p.tile([P, F], bf16, tag="rhs", name=f"rt_{b}_{j}_{kt}")
                        # split across DVE/ACT so neither paces the fc phase
                        eng = nc.vector.tensor_copy if kt < 2 else nc.scalar.copy
                        eng(out=rt[:], in_=xt[(kt, h)][:, jo : jo + F])
                        xt[("rt", kt)] = rt
                    oo = (j % 2) * F  # offset within the out-pair tile
                    f_all = [
                        ps_fc.tile([P, F], f32, tag="fc", name=f"f_ps_{b}_{j}_{m}")
                        for m in range(CT)
                    ]
                    for kt in range(CT):
                        for m in range(CT):
                            nc.tensor.matmul(
                                f_all[m][:],
                                lhsT=gst[:, kt, m * P : (m + 1) * P],
                                rhs=xt[("rt", kt)][:],
                                start=(kt == 0),
                                stop=(kt == CT - 1),
                            )
                    for m in range(CT):
                        f_ps = f_all[m]
                        if j % 2 == 0:
                            ot = out_p.tile(
                                [P, 2 * F], f32, tag="out", name=f"ot_{b}_{jj}_{m}"
                            )
                            xt[("out", jj, m)] = ot
                        else:
                            ot = xt[("out", jj, m)]
                        # y = (beta/Z)[c] * fc_raw + x, one fused DVE op
                        nc.vector.scalar_tensor_tensor(
                            out=ot[:, oo : oo + F],
                            in0=f_ps[:],
                            scalar=brs[m][:],
                            in1=xt[(m, h)][:, jo : jo + F],
                            op0=OP.mult,
                            op1=OP.add,
                        )
                        del f_ps
                        if j % 2 == 1:
                            sti = nc.gpsimd.dma_start(
                                out=y_d[
                                    b,
                                    m * P : (m + 1) * P,
                                    jj * 2 * F : (jj + 1) * 2 * F,
                                ],
                                in_=ot[:],
                            )
                            if first_store[b] is None:
                                first_store[b] = sti.ins
                                # stores come after the final xpose segment
                                add_dep_helper(
                                    sti.ins,
                                    last_xpose[0],
                                    reason="xbar segment: stores after all xposes",
                                )
    nc.finalize()
    return nc


def _get_nc():
    if "nc" not in _CACHE:
        _CACHE["nc"] = _build()
    return _CACHE["nc"]


def kernel(x: np.ndarray, beta: np.ndarray, **kw) -> np.ndarray:
    from concourse.bass_utils import run_bass_kernel_spmd

    x = np.ascontiguousarray(np.asarray(x, dtype=np.float32))
    beta = np.ascontiguousarray(np.asarray(beta, dtype=np.float32))
    assert x.shape == (B, C, 64, 64), x.shape

    xr = x.reshape(B, C, HW)
    in_maps = [
        {"x": np.ascontiguousarray(xr[i * BL : (i + 1) * BL]), "beta": beta}
        for i in range(NCORES)
    ]
    nc = _get_nc()
    res = run_bass_kernel_spmd(nc, in_maps, core_ids=list(range(NCORES)))
    out = np.concatenate([r["y"] for r in res.results], axis=0)
    return out.reshape(B, C, 64, 64).astype(np.float32)



# revision 2
# speedup vs baseline: 1.6942x; 1.6942x over previous
"""Channel-attention module (CAM) forward for Trainium2.

Per batch b:
    f1 = x[b].reshape(C, H*W)                      # [512, 4096]
    S  = f1 @ f1.T                                 # [512, 512]
    G  = softmax(S_max - S, axis=-1)               # == exp(S_min - S) / rowsum
    fc = G @ f1
    y[b] = beta * fc + x[b]

Sharding: data-parallel over batch B=16 across 8 NeuronCores (2 batches per
core), no cross-core communication.

Per-core dataflow (vs the bf16 baseline this halves DMA stores, removes the
DMA xbar transposes, and quarters PE matmul time):
  - All DMA rides SWDGE (Pool engine): f32 loads [128,1024], bf16 stores
    [128,2048].  The output DRAM tensor is bf16 (cast to f32 on host);
    rounding x to bf16 is ~1e-3 relative error, well inside tolerance.
  - Both GEMMs run as fp8e4 DoubleRow matmuls (two 128-deep k-blocks per
    instruction, 0.5 PE cycles/row).
  - f1^T is produced on the PE: fp8 transpose-mode matmuls write stride-2
    elements into PSUM (hardware requirement for fp8 transposes: element
    step 2, 4-byte-aligned base).  Each kh-pair bank is moved to SBUF with
    a single f32-bitcast DVE/ACT copy; S reads the stride-2 operands
    directly via strided APs.
  - Softmax: DVE row-min, ACT exp (bias=rowmin, scale=-1, fp8 out) with f32
    row-sum accum.  G stays unnormalized; beta/Z = beta*exp(-ln Z) (ACT) is
    folded into the epilogue scale.
  - G^T on the PE the same stride-2 way; fc lhsT reads it strided, rhs reads
    packed fp8 f1 (cast once from the resident f32 x tiles).
  - Epilogue: one fused DVE scalar_tensor_tensor per [128,512]:
    y_bf16 = (beta/Z)[c]*fc_raw + x_f32.
  - PE program order interleaves batch b's fc with batch b+1's transposes
    and S so the tensor engine never sits behind the DVE epilogue drain.
"""

import numpy as np

B, C, HW = 16, 512, 4096
NCORES = 8
BL = B // NCORES   # batches per core
P = 128
CT = C // P        # 4 c-blocks
NQ = 4             # load quarters (1024 cols each)
QW = HW // NQ      # 1024
NPAIR = 16         # kh-pairs (32 n-blocks of 128)
F = 512            # psum free dim / n-chunk
NCH = HW // F      # 8 n-chunks

# engine split knobs (tuned against the timeline sim)
CAST_DVE = {3}          # quarters q whose casts go to DVE instead of ACT
F1T_COPY_DVE_MOD = 2    # pair k copy goes to DVE when k % 2 == 0

_CACHE = {}


def _build():
    import concourse.bass as bass  # noqa: F401
    import concourse.mybir as mybir
    import concourse.tile as tile
    from concourse import bacc
    from concourse.masks import make_identity

    f32 = mybir.dt.float32
    bf16 = mybir.dt.bfloat16
    fp8 = mybir.dt.float8e4
    AF = mybir.ActivationFunctionType
    OP = mybir.AluOpType
    AX = mybir.AxisListType
    PM = mybir.MatmulPerfMode

    nc = bacc.Bacc("TRN2", target_bir_lowering=False, debug=False)
    x_d = nc.dram_tensor("x", [BL, C, HW], f32, kind="ExternalInput")
    beta_d = nc.dram_tensor("beta", [1], f32, kind="ExternalInput")
    y_d = nc.dram_tensor("y", [BL, C, HW], bf16, kind="ExternalOutput")

    with tile.TileContext(nc) as tc:
        with (
            tc.tile_pool(name="singles", bufs=1) as singles,
            tc.tile_pool(name="xf", bufs=26) as xf_p,        # [128,1024] f32
            tc.tile_pool(name="f1", bufs=2) as f1_p,         # [128,4,4096] fp8
            tc.tile_pool(name="f1t", bufs=8) as f1t_p,       # [128,2,1024] fp8 (stride-2)
            tc.tile_pool(name="gst", bufs=3) as gst_p,       # [128,2,1024] fp8 (stride-2)
            tc.tile_pool(name="e8", bufs=5) as e_p,          # [128,512] fp8
            tc.tile_pool(name="soft", bufs=16) as soft_p,    # [128,1] f32
            tc.tile_pool(name="obf", bufs=6) as out_p,       # [128,2048] bf16
            tc.tile_pool(name="ps_s", bufs=4, space="PSUM") as ps_s,
            tc.tile_pool(name="ps_x", bufs=2, space="PSUM") as ps_x,
            tc.tile_pool(name="ps_f", bufs=2, space="PSUM") as ps_f,
        ):
            beta_sb = singles.tile([P, 1], f32)
            nc.gpsimd.dma_start(out=beta_sb[:], in_=beta_d[:].to_broadcast([P, 1]))
            ident = singles.tile([P, P], fp8)
            make_identity(nc, ident[:])

            def emit_loads(b):
                """DMA loads + f32->fp8 casts for batch b."""
                xt = {}
                f1 = f1_p.tile([P, CT, HW], fp8, tag="f1", name=f"f1_{b}")
                for q in range(NQ):
                    for ct in range(CT):
                        t = xf_p.tile([P, QW], f32, tag="xf", name=f"x{b}_{ct}_{q}")
                        nc.gpsimd.dma_start(
                            out=t[:],
                            in_=x_d[b, ct * P : (ct + 1) * P, q * QW : (q + 1) * QW],
                        )
                        xt[(ct, q)] = t
                    for ct in range(CT):
                        eng = nc.vector.tensor_copy if q in CAST_DVE else nc.scalar.copy
                        eng(
                            out=f1[:, ct, q * QW : (q + 1) * QW],
                            in_=xt[(ct, q)][:],
                        )
                return xt, f1

            def emit_xpose_pair(b, f1, k):
                """PE-transpose kh-blocks 2k,2k+1 of f1 into a stride-2
                SBUF pair tile; returns the tile."""
                xp = ps_x.tile([P, 2, 2 * F], fp8, tag="xp", name=f"xp{b}_{k}")
                for i in range(2):
                    n0 = (2 * k + i) * P
                    for ct in range(CT):
                        nc.tensor.transpose(
                            xp[:, i, ct * 2 * P : ct * 2 * P + 2 * P : 2],
                            f1[:, ct, n0 : n0 + P],
                            ident[:],
                        )
                pair = f1t_p.tile([P, 2, 2 * F], fp8, tag="f1t", name=f"f1t{b}_{k}")
                eng = (
                    nc.vector.tensor_copy
                    if k % F1T_COPY_DVE_MOD == 0
                    else nc.scalar.copy
                )
                eng(
                    out=pair[:].rearrange("p a b -> p (a b)").bitcast(f32),
                    in_=xp[:].rearrange("p a b -> p (a b)").bitcast(f32),
                )
                return pair

            def emit_s_pair(b, s_ps, pairs, k):
                """S accumulation for kh-pair k (all 4 m-blocks)."""
                pair = pairs[k]
                rhs = pair[:, :, 0 : 2 * F : 2]
                for m in range(CT):
                    nc.tensor.matmul(
                        s_ps[m][:],
                        lhsT=pair[:, :, m * 2 * P : m * 2 * P + 2 * P : 2],
                        rhs=rhs,
                        start=(k == 0),
                        stop=(k == NPAIR - 1),
                        perf_mode=PM.DoubleRow,
                    )

            def emit_stats(b, s_ps):
                """Row stats + unnormalized exp (fp8) + beta/Z."""
                es, zs, brs = [], [], []
                for m in range(CT):
                    mn = soft_p.tile([P, 1], f32, tag="mn", name=f"mn{b}{m}")
                    nc.vector.tensor_reduce(
                        out=mn[:], in_=s_ps[m][:], axis=AX.X, op=OP.min
                    )
                    e = e_p.tile([P, F], fp8, tag="e", name=f"e{b}{m}")
                    z = soft_p.tile([P, 1], f32, tag="z", name=f"z{b}{m}")
                    nc.scalar.activation(
                        out=e[:],
                        in_=s_ps[m][:],
                        func=AF.Exp,
                        bias=mn[:],
                        scale=-1.0,
                        accum_out=z[:],
                    )
                    es.append(e)
                    zs.append(z)
                for m in range(CT):
                    lz = soft_p.tile([P, 1], f32, tag="lz", name=f"lz{b}{m}")
                    nc.scalar.activation(out=lz[:], in_=zs[m][:], func=AF.Ln)
                    br = soft_p.tile([P, 1], f32, tag="br", name=f"br{b}{m}")
                    nc.scalar.activation(out=br[:], in_=lz[:], func=AF.Exp, scale=-1.0)
                    nc.scalar.mul(out=br[:], in_=br[:], mul=beta_sb[:])
                    brs.append(br)
                return es, brs

            def emit_gt(b, es):
                """G^T via stride-2 PE transposes; two dt-pair tiles."""
                gst = []
                for q in range(2):
                    xp = ps_x.tile([P, 2, 2 * F], fp8, tag="xp", name=f"gx{b}_{q}")
                    for i in range(2):
                        dt = 2 * q + i
                        for m in range(CT):
                            nc.tensor.transpose(
                                xp[:, i, m * 2 * P : m * 2 * P + 2 * P : 2],
                                es[m][:, dt * P : (dt + 1) * P],
                                ident[:],
                            )
                    g = gst_p.tile([P, 2, 2 * F], fp8, tag="gst", name=f"gst{b}_{q}")
                    nc.scalar.copy(
                        out=g[:].rearrange("p a b -> p (a b)").bitcast(f32),
                        in_=xp[:].rearrange("p a b -> p (a b)").bitcast(f32),
                    )
                    gst.append(g)
                return gst

            def emit_fc_j(b, j, f1, gst, brs, xt, obf):
                """fc matmuls + fused epilogue + store for n-chunk j."""
                q, jo = j // 2, (j % 2) * F
                h, jj = j // 4, j % 4
                for m in range(CT):
                    f_ps = ps_f.tile([P, F], f32, tag="fc", name=f"f{b}_{j}_{m}")
                    for dq in range(2):
                        nc.tensor.matmul(
                            f_ps[:],
                            lhsT=gst[dq][:, :, m * 2 * P : m * 2 * P + 2 * P : 2],
                            rhs=f1[:, 2 * dq : 2 * dq + 2, j * F : (j + 1) * F],
                            start=(dq == 0),
                            stop=(dq == 1),
                            perf_mode=PM.DoubleRow,
                        )
                    if jj == 0:
                        ot = out_p.tile([P, 4 * F], bf16, tag="o", name=f"o{b}_{m}_{h}")
                        obf[(m, h)] = ot
                    else:
                        ot = obf[(m, h)]
                    nc.vector.scalar_tensor_tensor(
                        out=ot[:, jj * F : (jj + 1) * F],
                        in0=f_ps[:],
                        scalar=brs[m][:],
                        in1=xt[(m, q)][:, jo : jo + F],
                        op0=OP.mult,
                        op1=OP.add,
                    )
                    if jj == 3:
                        nc.gpsimd.dma_start(
                            out=y_d[b, m * P : (m + 1) * P, h * 4 * F : (h + 1) * 4 * F],
                            in_=ot[:],
                        )

            # ---------------- schedule ----------------
            xt0, f1_0 = emit_loads(0)
            state = {0: (xt0, f1_0)}

            # batch 0 prologue: transposes + S interleaved per pair
            s_ps = [
                ps_s.tile([P, F], f32, tag="s", name=f"s0_{m}") for m in range(CT)
            ]
            pairs = {}
            for k in range(NPAIR):
                pairs[k] = emit_xpose_pair(0, f1_0, k)
                emit_s_pair(0, s_ps, pairs, k)

            for b in range(BL):
                es, brs = emit_stats(b, s_ps)
                if b + 1 < BL:
                    state[b + 1] = emit_loads(b + 1)
                gst = emit_gt(b, es)

                xt, f1 = state[b]
                obf = {}
                if b + 1 < BL:
                    # interleave fc(b) with xpose+S of b+1
                    _, f1n = state[b + 1]
                    s_ps = [
                        ps_s.tile([P, F], f32, tag="s", name=f"s{b + 1}_{m}")
                        for m in range(CT)
                    ]
                    pairs = {}
                    for j in range(NCH):
                        for k in (2 * j, 2 * j + 1):
                            pairs[k] = emit_xpose_pair(b + 1, f1n, k)
                        emit_fc_j(b, j, f1, gst, brs, xt, obf)
                        for k in (2 * j - 2, 2 * j - 1):
                            if k >= 0:
                                emit_s_pair(b + 1, s_ps, pairs, k)
                    emit_s_pair(b + 1, s_ps, pairs, NPAIR - 2)
                    emit_s_pair(b + 1, s_ps, pairs, NPAIR - 1)
                else:
                    for j in range(NCH):
                        emit_fc_j(b, j, f1, gst, brs, xt, obf)

    nc.finalize()
    return nc


def _get_nc():
    if "nc" not in _CACHE:
        _CACHE["nc"] = _build()
    return _CACHE["nc"]


def kernel(x: np.ndarray, beta: np.ndarray, **kw) -> np.ndarray:
    from concourse.bass_utils import run_bass_kernel_spmd

    x = np.ascontiguousarray(np.asarray(x, dtype=np.float32))
    beta = np.ascontiguousarray(np.asarray(beta, dtype=np.float32))
    assert x.shape == (B, C, 64, 64), x.shape

    xr = x.reshape(B, C, HW)
    in_maps = [
        {"x": np.ascontiguousarray(xr[i * BL : (i + 1) * BL]), "beta": beta}
        for i in range(NCORES)
    ]
    nc = _get_nc()
    res = run_bass_kernel_spmd(nc, in_maps, core_ids=list(range(NCORES)))
    out = np.concatenate(
        [np.asarray(r["y"], dtype=np.float32) for r in res.results], axis=0
    )
    return out.reshape(B, C, 64, 64)


# revision 64
# speedup vs baseline: 2.2875x; 1.3502x over previous
"""Channel-attention module (CAM) forward for Trainium2.

Per batch b:
    f1 = x[b].reshape(C, H*W)                      # [512, 4096]
    S  = f1 @ f1.T                                 # [512, 512]
    G  = softmax(S_max - S, axis=-1)               # == exp(S_min - S) / rowsum
    fc = G @ f1
    y[b] = beta * fc + x[b]

Sharding: data-parallel over batch B=16 across 8 NeuronCores (2 batches per
core), no cross-core communication.

Per-core dataflow (vs the bf16 baseline this halves DMA stores, removes the
DMA xbar transposes, and quarters PE matmul time):
  - All DMA rides SWDGE (Pool engine): f32 loads [128,1024], bf16 stores
    [128,1024].  The output DRAM tensor is bf16 (cast to f32 on host);
    rounding x to bf16 is ~1e-3 relative error, well inside tolerance.
  - Both GEMMs run as fp8e4 DoubleRow matmuls (two 128-deep k-blocks per
    instruction, 0.5 PE cycles/row).
  - f1^T is produced on the PE: fp8 transpose-mode matmuls write stride-2
    elements into PSUM (hardware requires element step 2 with 4-byte-aligned
    base for fp8 transposes).  Each kh-pair bank moves to SBUF with a single
    f32-bitcast copy; S reads the stride-2 operands via strided APs.
  - Softmax: DVE row-min, ACT exp (bias=rowmin, scale=-1, fp8 out) with f32
    row-sum accum; DVE reciprocal gives beta/Z (ACT stays on one act table).
    e2 = (64*beta/Z)*e is folded into G, so every epilogue variant is just
    (64*x + fc64)/64.
  - G^T on the PE the same stride-2 way; fc lhsT reads it strided, rhs reads
    packed fp8 f1 (cast once from the resident x tiles, which are f32r so
    the PE can matmul them directly).
  - Epilogue: batch 0 uses a fused DVE scalar_tensor_tensor
    (psum/64 + x -> bf16) while ACT casts batch 1's x; batch 1's tail
    preloads 64*x into psum via an f32r identity matmul (PE is idle there),
    accumulates fc64 on top, and drains with 1/64 psum->bf16 copies
    alternating between DVE and ACT, with psum tiles drawn alternately from
    the S pool (free by then) and the fc pool so copies overlap.
  - PE program order interleaves batch b's fc with batch b+1's transposes
    and S so the tensor engine never sits behind the epilogue drain.
"""

import numpy as np

B, C, HW = 16, 512, 4096
NCORES = 8
BL = B // NCORES   # batches per core
P = 128
CT = C // P        # 4 c-blocks
NQ = 4             # load quarters (1024 cols each)
QW = HW // NQ      # 1024
NPAIR = 16         # kh-pairs (32 n-blocks of 128)
F = 512            # psum free dim / n-chunk
NCH = HW // F      # 8 n-chunks

# engine-split knobs (tuned against the timeline sim)
CAST_DVE = {0: {0, 1, 2, 3}, 1: {0, 1}}  # per-batch quarters cast on DVE
F1T_DVE = {0: lambda k: True, 1: lambda k: False}  # f1t copy on DVE?
# per (batch, m-group) epilogue mode: "stt" fused DVE; "pre_dve"/"pre_act"
# 64*x psum preload + 1/64 copy on that engine
EPI_MODE = {
    0: ["stt"] * 8,
    1: ["pre_dve", "pre_act"] * 4,
}

_CACHE = {}


def _build():
    import concourse.bass as bass  # noqa: F401
    import concourse.mybir as mybir
    import concourse.tile as tile
    from concourse import bacc
    from concourse.masks import make_identity

    f32 = mybir.dt.float32
    f32r = mybir.dt.float32r
    bf16 = mybir.dt.bfloat16
    fp8 = mybir.dt.float8e4
    AF = mybir.ActivationFunctionType
    OP = mybir.AluOpType
    AX = mybir.AxisListType
    PM = mybir.MatmulPerfMode

    nc = bacc.Bacc("TRN2", target_bir_lowering=False, debug=False)
    x_d = nc.dram_tensor("x", [BL, C, HW], f32, kind="ExternalInput")
    beta_d = nc.dram_tensor("beta", [1], f32, kind="ExternalInput")
    y_d = nc.dram_tensor("y", [BL, C, HW], bf16, kind="ExternalOutput")

    with tile.TileContext(nc) as tc:
        with (
            tc.tile_pool(name="singles", bufs=1) as singles,
            tc.tile_pool(name="xf", bufs=26) as xf_p,        # [128,1024] f32r
            tc.tile_pool(name="f1", bufs=2) as f1_p,         # [128,4,4096] fp8
            tc.tile_pool(name="f1t", bufs=18) as f1t_p,      # [128,2,1024] fp8 (stride-2)
            tc.tile_pool(name="gst", bufs=3) as gst_p,       # [128,2,1024] fp8 (stride-2)
            tc.tile_pool(name="e8", bufs=10) as e_p,         # [128,512] fp8
            tc.tile_pool(name="soft", bufs=16) as soft_p,    # [128,1] f32
            tc.tile_pool(name="obf", bufs=10) as out_p,      # [128,1024] bf16
            tc.tile_pool(name="ps_s", bufs=4, space="PSUM") as ps_s,
            tc.tile_pool(name="ps_x", bufs=2, space="PSUM") as ps_x,
            tc.tile_pool(name="ps_f", bufs=2, space="PSUM") as ps_f,
        ):
            beta_sb = singles.tile([P, 1], f32)
            nc.gpsimd.dma_start(out=beta_sb[:], in_=beta_d[:].to_broadcast([P, 1]))
            beta64 = singles.tile([P, 1], f32)
            nc.scalar.mul(out=beta64[:], in_=beta_sb[:], mul=64.0)
            ident = singles.tile([P, P], fp8)
            make_identity(nc, ident[:])
            # 64*I in f32r for the fc psum x-preload (f32r operands must come
            # from an f32r-rounding producer, hence the ACT copy)
            identf = singles.tile([P, P], f32)
            nc.gpsimd.memset(identf[:], 0.0)
            nc.gpsimd.affine_select(
                out=identf[:],
                in_=identf[:],
                compare_op=OP.not_equal,
                fill=64.0,
                base=0,
                pattern=[[-1, P]],
                channel_multiplier=1,
            )
            ident64 = singles.tile([P, P], f32r)
            nc.scalar.copy(out=ident64[:], in_=identf[:])

            def emit_loads_q(b, f1, xt, q):
                """DMA loads + f32->fp8 casts for quarter q of batch b.
                x tiles are f32r so the fc preload matmul can read them."""
                for ct in range(CT):
                    t = xf_p.tile([P, QW], f32r, tag="xf", name=f"x{b}_{ct}_{q}")
                    nc.gpsimd.dma_start(
                        out=t[:],
                        in_=x_d[
                            b, ct * P : (ct + 1) * P, q * QW : (q + 1) * QW
                        ].bitcast(f32r),
                    )
                    xt[(ct, q)] = t
                for ct in range(CT):
                    eng = (
                        nc.vector.tensor_copy
                        if q in CAST_DVE[b]
                        else nc.scalar.copy
                    )
                    eng(
                        out=f1[:, ct, q * QW : (q + 1) * QW],
                        in_=xt[(ct, q)][:].bitcast(f32),
                    )

            def emit_xpose_pair(b, f1, k):
                """PE-transpose kh-blocks 2k,2k+1 of f1 into a stride-2
                SBUF pair tile; returns the tile."""
                xp = ps_x.tile([P, 2, 2 * F], fp8, tag="xp", name=f"xp{b}_{k}")
                for i in range(2):
                    n0 = (2 * k + i) * P
                    for ct in range(CT):
                        nc.tensor.transpose(
                            xp[:, i, ct * 2 * P : ct * 2 * P + 2 * P : 2],
                            f1[:, ct, n0 : n0 + P],
                            ident[:],
                        )
                pair = f1t_p.tile([P, 2, 2 * F], fp8, tag="f1t", name=f"f1t{b}_{k}")
                eng = nc.vector.tensor_copy if F1T_DVE[b](k) else nc.scalar.copy
                eng(
                    out=pair[:].rearrange("p a b -> p (a b)").bitcast(f32),
                    in_=xp[:].rearrange("p a b -> p (a b)").bitcast(f32),
                )
                return pair

            def emit_s_pair(b, s_ps, pairs, k):
                """S accumulation for kh-pair k (all 4 m-blocks)."""
                pair = pairs[k]
                rhs = pair[:, :, 0 : 2 * F : 2]
                for m in range(CT):
                    nc.tensor.matmul(
                        s_ps[m][:],
                        lhsT=pair[:, :, m * 2 * P : m * 2 * P + 2 * P : 2],
                        rhs=rhs,
                        start=(k == 0),
                        stop=(k == NPAIR - 1),
                        perf_mode=PM.DoubleRow,
                    )

            def emit_stats(b, s_ps):
                """Row stats, unnormalized exp (fp8), then e2 = (64*beta/Z)*e.

                Separate loops per stage: a per-m reduce->exp->recip order
                would head-of-line-block the in-order DVE queue on each exp's
                accumulator, serializing the whole chain."""
                es, zs, brs, e2s, mns = {}, {}, {}, {}, {}
                for m in range(CT):
                    mns[m] = soft_p.tile([P, 1], f32, tag="mn", name=f"mn{b}{m}")
                    nc.vector.tensor_reduce(
                        out=mns[m][:], in_=s_ps[m][:], axis=AX.X, op=OP.min
                    )
                for m in range(CT):
                    es[m] = e_p.tile([P, F], fp8, tag="e", name=f"e{b}{m}")
                    zs[m] = soft_p.tile([P, 1], f32, tag="z", name=f"z{b}{m}")
                    nc.scalar.activation(
                        out=es[m][:],
                        in_=s_ps[m][:],
                        func=AF.Exp,
                        bias=mns[m][:],
                        scale=-1.0,
                        accum_out=zs[m][:],
                    )
                for m in range(CT):
                    br = soft_p.tile([P, 1], f32, tag="br", name=f"br{b}{m}")
                    nc.vector.reciprocal(out=br[:], in_=zs[m][:])
                    nc.vector.tensor_mul(br[:], br[:], beta64[:])
                    brs[m] = br
                for m in range(CT):
                    e2 = e_p.tile([P, F], fp8, tag="e", name=f"e2_{b}{m}")
                    nc.vector.tensor_scalar_mul(e2[:], es[m][:], brs[m][:])
                    e2s[m] = e2
                return e2s

            def emit_gt(b, e2s):
                """G^T via stride-2 PE transposes; two dt-pair tiles."""
                gst = []
                for q in range(2):
                    xp = ps_x.tile([P, 2, 2 * F], fp8, tag="xp", name=f"gx{b}_{q}")
                    for i in range(2):
                        dt = 2 * q + i
                        for m in range(CT):
                            nc.tensor.transpose(
                                xp[:, i, m * 2 * P : m * 2 * P + 2 * P : 2],
                                e2s[m][:, dt * P : (dt + 1) * P],
                                ident[:],
                            )
                    g = gst_p.tile([P, 2, 2 * F], fp8, tag="gst", name=f"gst{b}_{q}")
                    nc.scalar.copy(
                        out=g[:].rearrange("p a b -> p (a b)").bitcast(f32),
                        in_=xp[:].rearrange("p a b -> p (a b)").bitcast(f32),
                    )
                    gst.append(g)
                return gst

            def emit_fc_m(b, h, m, f1, gst, xt, g):
                """fc matmuls + epilogue + 2 stores for (m, h).  G carries
                the 64*beta/Z scale, so every mode computes
                y = (64*x + fc64)/64."""
                mode = EPI_MODE[b][g]
                for pj in range(2):
                    ot = out_p.tile(
                        [P, 2 * F], bf16, tag="o", name=f"o{b}_{m}_{h}_{pj}"
                    )
                    for jj in range(2 * pj, 2 * pj + 2):
                        j = h * 4 + jj
                        q, qo = j // 2, (j % 2) * F
                        # the tail's preload modes draw psum alternately from
                        # the S pool (free once exps(b) are done) and the fc
                        # pool, so the two copy engines can overlap
                        pool = ps_s if mode != "stt" and jj % 2 else ps_f
                        f_ps = pool.tile(
                            [P, F], f32, tag="s" if pool is ps_s else "fc",
                            name=f"f{b}_{j}_{m}",
                        )
                        if mode != "stt":
                            nc.tensor.matmul(
                                f_ps[:],
                                lhsT=ident64[:],
                                rhs=xt[(m, q)][:, qo : qo + F],
                                start=True,
                                stop=False,
                            )
                        for dq in range(2):
                            nc.tensor.matmul(
                                f_ps[:],
                                lhsT=gst[dq][:, :, m * 2 * P : m * 2 * P + 2 * P : 2],
                                rhs=f1[:, 2 * dq : 2 * dq + 2, j * F : (j + 1) * F],
                                start=(mode == "stt" and dq == 0),
                                stop=(dq == 1),
                                perf_mode=PM.DoubleRow,
                            )
                        osl = ot[:, (jj % 2) * F : (jj % 2) * F + F]
                        if mode == "stt":
                            nc.vector.scalar_tensor_tensor(
                                out=osl,
                                in0=f_ps[:],
                                scalar=1.0 / 64.0,
                                in1=xt[(m, q)][:, qo : qo + F].bitcast(f32),
                                op0=OP.mult,
                                op1=OP.add,
                            )
                        elif mode == "pre_dve":
                            nc.vector.tensor_scalar_mul(osl, f_ps[:], 1.0 / 64.0)
                        else:
                            nc.scalar.mul(out=osl, in_=f_ps[:], mul=1.0 / 64.0)
                    c0 = h * 4 * F + pj * 2 * F
                    # alternate stores between SWDGE (Pool descgen) and the
                    # otherwise-idle SP HWDGE path so descriptor generation
                    # never paces the store drain
                    eng = nc.sync if pj else nc.gpsimd
                    eng.dma_start(
                        out=y_d[b, m * P : (m + 1) * P, c0 : c0 + 2 * F],
                        in_=ot[:],
                    )

            # ---------------- schedule ----------------
            # batch 0 prologue: loads/casts/xpose/S interleaved per quarter
            xt0 = {}
            f1_0 = f1_p.tile([P, CT, HW], fp8, tag="f1", name="f1_0")
            state = {0: (xt0, f1_0)}
            s_ps = {
                m: ps_s.tile([P, F], f32, tag="s", name=f"s0_{m}")
                for m in range(CT)
            }
            pairs = {}
            for q in range(NQ):
                emit_loads_q(0, f1_0, xt0, q)
                for k in range(4 * q, 4 * q + 4):
                    pairs[k] = emit_xpose_pair(0, f1_0, k)
                    emit_s_pair(0, s_ps, pairs, k)

            for b in range(BL):
                e2s = emit_stats(b, s_ps)
                gst = emit_gt(b, e2s)
                xt, f1 = state[b]

                if b + 1 < BL:
                    xtn = {}
                    f1n = f1_p.tile([P, CT, HW], fp8, tag="f1", name=f"f1_{b + 1}")
                    state[b + 1] = (xtn, f1n)
                    for q in range(NQ):
                        emit_loads_q(b + 1, f1n, xtn, q)
                    # interleave fc(b) m-groups with xpose + S of b+1
                    s_ps_n = {
                        m: ps_s.tile([P, F], f32, tag="s", name=f"s{b + 1}_{m}")
                        for m in range(CT)
                    }
                    pairs_n = {}
                    # pairs lag one m-group so fc(b) m=0 is not stuck behind
                    # xpose(b+1) (which waits on loads(b+1)) in PE program
                    # order; the last group takes four pairs so S(b+1)
                    # finishes with fc(b)
                    grp_pairs = [[], [0, 1], [2, 3], [4, 5], [6, 7],
                                 [8, 9], [10, 11], [12, 13, 14, 15]]
                    g = 0
                    for h in range(2):
                        for m in range(CT):
                            for k in grp_pairs[g]:
                                pairs_n[k] = emit_xpose_pair(b + 1, f1n, k)
                            emit_fc_m(b, h, m, f1, gst, xt, g)
                            for k in grp_pairs[g]:
                                emit_s_pair(b + 1, s_ps_n, pairs_n, k)
                            g += 1
                    s_ps, pairs = s_ps_n, pairs_n
                else:
                    g = 0
                    for h in range(2):
                        for m in range(CT):
                            emit_fc_m(b, h, m, f1, gst, xt, g)
                            g += 1

    nc.finalize()
    return nc


def _get_nc():
    if "nc" not in _CACHE:
        _CACHE["nc"] = _build()
    return _CACHE["nc"]


def kernel(x: np.ndarray, beta: np.ndarray, **kw) -> np.ndarray:
    from concourse.bass_utils import run_bass_kernel_spmd

    x = np.ascontiguousarray(np.asarray(x, dtype=np.float32))
    beta = np.ascontiguousarray(np.asarray(beta, dtype=np.float32))
    assert x.shape == (B, C, 64, 64), x.shape

    xr = x.reshape(B, C, HW)
    in_maps = [
        {"x": np.ascontiguousarray(xr[i * BL : (i + 1) * BL]), "beta": beta}
        for i in range(NCORES)
    ]
    nc = _get_nc()
    res = run_bass_kernel_spmd(nc, in_maps, core_ids=list(range(NCORES)))
    out = np.concatenate(
        [np.asarray(r["y"], dtype=np.float32) for r in res.results], axis=0
    )
    return out.reshape(B, C, 64, 64)


# revision 88
# speedup vs baseline: 2.4589x; 1.0749x over previous
"""Channel-attention module (CAM) forward for Trainium2.

Per batch b:
    f1 = x[b].reshape(C, H*W)                      # [512, 4096]
    S  = f1 @ f1.T                                 # [512, 512]
    G  = softmax(S_max - S, axis=-1)               # == exp(S_min - S) / rowsum
    fc = G @ f1
    y[b] = beta * fc + x[b]

Sharding: data-parallel over batch B=16 across 8 NeuronCores (2 batches per
core), no cross-core communication.

Per-core dataflow (vs the bf16 baseline this halves DMA stores, removes the
DMA xbar transposes, and quarters PE matmul time):
  - All DMA rides SWDGE (Pool engine): f32 loads [128,1024], bf16 stores
    [128,1024].  The output DRAM tensor is bf16 (cast to f32 on host);
    rounding x to bf16 is ~1e-3 relative error, well inside tolerance.
  - Both GEMMs run as fp8e4 DoubleRow matmuls (two 128-deep k-blocks per
    instruction, 0.5 PE cycles/row).
  - f1^T is produced on the PE: fp8 transpose-mode matmuls write stride-2
    elements into PSUM (hardware requires element step 2 with 4-byte-aligned
    base for fp8 transposes).  Each kh-pair bank moves to SBUF with a single
    f32-bitcast copy; S reads the stride-2 operands via strided APs.
  - Softmax: DVE row-min, ACT exp (bias=rowmin, scale=-1, fp8 out) with f32
    row-sum accum; DVE reciprocal gives beta/Z (ACT stays on one act table).
    e2 = (64*beta/Z)*e is folded into G, so every epilogue variant is just
    (64*x + fc64)/64.
  - G^T on the PE the same stride-2 way; fc lhsT reads it strided, rhs reads
    packed fp8 f1 (cast once from the resident x tiles, which are f32r so
    the PE can matmul them directly).
  - Epilogue: batch 0 uses a fused DVE scalar_tensor_tensor
    (psum/64 + x -> bf16) while ACT casts batch 1's x; batch 1's tail
    preloads 64*x into psum via an f32r identity matmul (PE is idle there),
    accumulates fc64 on top, and drains with 1/64 psum->bf16 copies
    alternating between DVE and ACT, with psum tiles drawn alternately from
    the S pool (free by then) and the fc pool so copies overlap.
  - PE program order interleaves batch b's fc with batch b+1's transposes
    and S so the tensor engine never sits behind the epilogue drain.
"""

import numpy as np

B, C, HW = 16, 512, 4096
NCORES = 8
BL = B // NCORES   # batches per core
P = 128
CT = C // P        # 4 c-blocks
NQ = 4             # load quarters (1024 cols each)
QW = HW // NQ      # 1024
NPAIR = 16         # kh-pairs (32 n-blocks of 128)
F = 512            # psum free dim / n-chunk
NCH = HW // F      # 8 n-chunks

# engine-split knobs (tuned against the timeline sim)
CAST_DVE = {0: {0, 1, 2, 3}, 1: {0, 1}}  # per-batch quarters cast on DVE
F1T_DVE = {0: lambda k: k % 2 == 0, 1: lambda k: False}  # f1t copy on DVE?
# per (batch, m-group) epilogue mode: "stt" = fused DVE (psum/64 + x);
# "pre" = 64*x psum preload + 1/64 psum->bf16 copies alternating DVE/ACT
# per chunk (drains a 2-bank ring at 2x)
EPI_MODE = {
    0: ["stt"] * 8,
    1: ["pre"] * 8,
}
SPLIT_Q = 1   # batch-0 quarters < SPLIT_Q alternate SWDGE/HWDGE loads
EARLY_STATS = False  # emit stats(b+1) before the last fc(b) group's epis
GRP_PAIRS = [[], [0, 1], [2, 3], [4, 5], [6, 7],
             [8, 9], [10, 11], [12, 13, 14, 15]]
CAST_Q2_G, CAST_Q3_G = 4, 6

_CACHE = {}


def _build():
    import concourse.bass as bass  # noqa: F401
    import concourse.mybir as mybir
    import concourse.tile as tile
    from concourse import bacc
    from concourse.masks import make_identity

    f32 = mybir.dt.float32
    f32r = mybir.dt.float32r
    bf16 = mybir.dt.bfloat16
    fp8 = mybir.dt.float8e4
    AF = mybir.ActivationFunctionType
    OP = mybir.AluOpType
    AX = mybir.AxisListType
    PM = mybir.MatmulPerfMode

    nc = bacc.Bacc("TRN2", target_bir_lowering=False, debug=False)
    x_d = nc.dram_tensor("x", [BL, C, HW], f32, kind="ExternalInput")
    beta_d = nc.dram_tensor("beta", [1], f32, kind="ExternalInput")
    y_d = nc.dram_tensor("y", [BL, C, HW], bf16, kind="ExternalOutput")

    with tile.TileContext(nc) as tc:
        with (
            tc.tile_pool(name="singles", bufs=1) as singles,
            tc.tile_pool(name="xf", bufs=26) as xf_p,        # [128,1024] f32r
            tc.tile_pool(name="f1", bufs=2) as f1_p,         # [128,4,4096] fp8
            tc.tile_pool(name="f1t", bufs=18) as f1t_p,      # [128,2,1024] fp8 (stride-2)
            tc.tile_pool(name="gst", bufs=3) as gst_p,       # [128,2,1024] fp8 (stride-2)
            tc.tile_pool(name="e8", bufs=10) as e_p,         # [128,512] fp8
            tc.tile_pool(name="soft", bufs=16) as soft_p,    # [128,1] f32
            tc.tile_pool(name="obf", bufs=10) as out_p,      # [128,1024] bf16
            tc.tile_pool(name="ps_s", bufs=4, space="PSUM") as ps_s,
            tc.tile_pool(name="ps_x", bufs=2, space="PSUM") as ps_x,
            tc.tile_pool(name="ps_f", bufs=2, space="PSUM") as ps_f,
        ):
            beta_sb = singles.tile([P, 1], f32)
            nc.gpsimd.dma_start(out=beta_sb[:], in_=beta_d[:].to_broadcast([P, 1]))
            beta64 = singles.tile([P, 1], f32)
            nc.scalar.mul(out=beta64[:], in_=beta_sb[:], mul=64.0)
            ident = singles.tile([P, P], fp8)
            make_identity(nc, ident[:])
            # 64*I in f32r for the fc psum x-preload (f32r operands must come
            # from an f32r-rounding producer, hence the ACT copy)
            identf = singles.tile([P, P], f32)
            nc.gpsimd.memset(identf[:], 0.0)
            nc.gpsimd.affine_select(
                out=identf[:],
                in_=identf[:],
                compare_op=OP.not_equal,
                fill=64.0,
                base=0,
                pattern=[[-1, P]],
                channel_multiplier=1,
            )
            ident64 = singles.tile([P, P], f32r)
            nc.scalar.copy(out=ident64[:], in_=identf[:])

            def emit_dma_q(b, xt, q, split=False):
                """DMA loads for quarter q of batch b.  x tiles are f32r so
                the fc preload matmul can read them.  With split=True,
                alternate SWDGE/HWDGE so the cold-start descgen pipelines."""
                for ct in range(CT):
                    t = xf_p.tile([P, QW], f32r, tag="xf", name=f"x{b}_{ct}_{q}")
                    eng = nc.sync if split and ct % 2 == 0 else nc.gpsimd
                    eng.dma_start(
                        out=t[:],
                        in_=x_d[
                            b, ct * P : (ct + 1) * P, q * QW : (q + 1) * QW
                        ].bitcast(f32r),
                    )
                    xt[(ct, q)] = t

            def emit_casts_q(b, f1, xt, q):
                """f32->fp8 casts for quarter q of batch b."""
                for ct in range(CT):
                    eng = (
                        nc.vector.tensor_copy
                        if q in CAST_DVE[b]
                        else nc.scalar.copy
                    )
                    eng(
                        out=f1[:, ct, q * QW : (q + 1) * QW],
                        in_=xt[(ct, q)][:].bitcast(f32),
                    )

            def emit_loads_q(b, f1, xt, q):
                emit_dma_q(b, xt, q)
                emit_casts_q(b, f1, xt, q)

            def emit_xpose_pair(b, f1, k):
                """PE-transpose kh-blocks 2k,2k+1 of f1 into a stride-2
                SBUF pair tile; returns the tile."""
                xp = ps_x.tile([P, 2, 2 * F], fp8, tag="xp", name=f"xp{b}_{k}")
                for i in range(2):
                    n0 = (2 * k + i) * P
                    for ct in range(CT):
                        nc.tensor.transpose(
                            xp[:, i, ct * 2 * P : ct * 2 * P + 2 * P : 2],
                            f1[:, ct, n0 : n0 + P],
                            ident[:],
                        )
                pair = f1t_p.tile([P, 2, 2 * F], fp8, tag="f1t", name=f"f1t{b}_{k}")
                eng = nc.vector.tensor_copy if F1T_DVE[b](k) else nc.scalar.copy
                eng(
                    out=pair[:].rearrange("p a b -> p (a b)").bitcast(f32),
                    in_=xp[:].rearrange("p a b -> p (a b)").bitcast(f32),
                )
                return pair

            def emit_s_pair(b, s_ps, pairs, k):
                """S accumulation for kh-pair k (all 4 m-blocks)."""
                pair = pairs[k]
                rhs = pair[:, :, 0 : 2 * F : 2]
                for m in range(CT):
                    nc.tensor.matmul(
                        s_ps[m][:],
                        lhsT=pair[:, :, m * 2 * P : m * 2 * P + 2 * P : 2],
                        rhs=rhs,
                        start=(k == 0),
                        stop=(k == NPAIR - 1),
                        perf_mode=PM.DoubleRow,
                    )

            def emit_stats(b, s_ps):
                """Row stats, unnormalized exp (fp8), then e2 = (64*beta/Z)*e.

                Separate loops per stage: a per-m reduce->exp->recip order
                would head-of-line-block the in-order DVE queue on each exp's
                accumulator, serializing the whole chain."""
                es, zs, brs, e2s, mns = {}, {}, {}, {}, {}
                for m in range(CT):
                    mns[m] = soft_p.tile([P, 1], f32, tag="mn", name=f"mn{b}{m}")
                    nc.vector.tensor_reduce(
                        out=mns[m][:], in_=s_ps[m][:], axis=AX.X, op=OP.min
                    )
                for m in range(CT):
                    es[m] = e_p.tile([P, F], fp8, tag="e", name=f"e{b}{m}")
                    zs[m] = soft_p.tile([P, 1], f32, tag="z", name=f"z{b}{m}")
                    nc.scalar.activation(
                        out=es[m][:],
                        in_=s_ps[m][:],
                        func=AF.Exp,
                        bias=mns[m][:],
                        scale=-1.0,
                        accum_out=zs[m][:],
                    )
                for m in range(CT):
                    br = soft_p.tile([P, 1], f32, tag="br", name=f"br{b}{m}")
                    nc.vector.reciprocal(out=br[:], in_=zs[m][:])
                    nc.vector.tensor_mul(br[:], br[:], beta64[:])
                    brs[m] = br
                for m in range(CT):
                    e2 = e_p.tile([P, F], fp8, tag="e", name=f"e2_{b}{m}")
                    nc.vector.tensor_scalar_mul(e2[:], es[m][:], brs[m][:])
                    e2s[m] = e2
                return e2s

            def emit_gt(b, e2s):
                """G^T via stride-2 PE transposes, filled per m-pair so the
                first fc m-groups are not gated on the last e2 blocks."""
                gst = [
                    gst_p.tile([P, 2, 2 * F], fp8, tag="gst", name=f"gst{b}_{q}")
                    for q in range(2)
                ]
                for mp in range(2):
                    for q in range(2):
                        xp = ps_x.tile(
                            [P, 2, F], fp8, tag="xp", name=f"gx{b}_{q}_{mp}"
                        )
                        for i in range(2):
                            dt = 2 * q + i
                            for mm in range(2):
                                m = 2 * mp + mm
                                nc.tensor.transpose(
                                    xp[:, i, mm * 2 * P : mm * 2 * P + 2 * P : 2],
                                    e2s[m][:, dt * P : (dt + 1) * P],
                                    ident[:],
                                )
                        nc.scalar.copy(
                            out=gst[q][:, :, mp * F : (mp + 1) * F].bitcast(f32),
                            in_=xp[:].bitcast(f32),
                        )
                return gst

            def emit_fc_m(b, h, m, f1, gst, xt, g):
                """fc matmuls + epilogue + 2 stores for (m, h).  G carries
                the 64*beta/Z scale, so every mode computes
                y = (64*x + fc64)/64."""
                mode = EPI_MODE[b][g]
                for pj in range(2):
                    ot = out_p.tile(
                        [P, 2 * F], bf16, tag="o", name=f"o{b}_{m}_{h}_{pj}"
                    )
                    for jj in range(2 * pj, 2 * pj + 2):
                        j = h * 4 + jj
                        q, qo = j // 2, (j % 2) * F
                        # the last batch's preload groups draw psum
                        # alternately from the S pool (free once its exps are
                        # done) and the fc pool; batch 0 must stay on ps_f
                        # (the S pool holds batch 1's accumulators - sharing
                        # it would deadlock the in-order PE stream)
                        pool = ps_s if b == BL - 1 and mode == "pre" and jj % 2 else ps_f
                        f_ps = pool.tile(
                            [P, F], f32, tag="s" if pool is ps_s else "fc",
                            name=f"f{b}_{j}_{m}",
                        )
                        if mode != "stt":
                            nc.tensor.matmul(
                                f_ps[:],
                                lhsT=ident64[:],
                                rhs=xt[(m, q)][:, qo : qo + F],
                                start=True,
                                stop=False,
                            )
                        for dq in range(2):
                            nc.tensor.matmul(
                                f_ps[:],
                                lhsT=gst[dq][:, :, m * 2 * P : m * 2 * P + 2 * P : 2],
                                rhs=f1[:, 2 * dq : 2 * dq + 2, j * F : (j + 1) * F],
                                start=(mode == "stt" and dq == 0),
                                stop=(dq == 1),
                                perf_mode=PM.DoubleRow,
                            )
                        osl = ot[:, (jj % 2) * F : (jj % 2) * F + F]
                        if mode == "stt":
                            nc.vector.scalar_tensor_tensor(
                                out=osl,
                                in0=f_ps[:],
                                scalar=1.0 / 64.0,
                                in1=xt[(m, q)][:, qo : qo + F].bitcast(f32),
                                op0=OP.mult,
                                op1=OP.add,
                            )
                        elif jj % 2:
                            nc.scalar.mul(out=osl, in_=f_ps[:], mul=1.0 / 64.0)
                        else:
                            nc.vector.tensor_scalar_mul(osl, f_ps[:], 1.0 / 64.0)
                    c0 = h * 4 * F + pj * 2 * F
                    # alternate stores between SWDGE (Pool descgen) and the
                    # otherwise-idle SP HWDGE path so descriptor generation
                    # never paces the store drain
                    eng = nc.sync if pj else nc.gpsimd
                    eng.dma_start(
                        out=y_d[b, m * P : (m + 1) * P, c0 : c0 + 2 * F],
                        in_=ot[:],
                    )

            # ---------------- schedule ----------------
            # batch 0 prologue: loads/casts/xpose/S interleaved per quarter
            xt0 = {}
            f1_0 = f1_p.tile([P, CT, HW], fp8, tag="f1", name="f1_0")
            state = {0: (xt0, f1_0)}
            s_ps = {
                m: ps_s.tile([P, F], f32, tag="s", name=f"s0_{m}")
                for m in range(CT)
            }
            pairs = {}
            for q in range(NQ):
                emit_dma_q(0, xt0, q, split=(q < SPLIT_Q))
                emit_casts_q(0, f1_0, xt0, q)
                for k in range(4 * q, 4 * q + 4):
                    pairs[k] = emit_xpose_pair(0, f1_0, k)
                    emit_s_pair(0, s_ps, pairs, k)

            e2s = None
            for b in range(BL):
                if e2s is None:
                    e2s = emit_stats(b, s_ps)
                gst = emit_gt(b, e2s)
                xt, f1 = state[b]
                e2s = None

                if b + 1 < BL:
                    xtn = {}
                    f1n = f1_p.tile([P, CT, HW], fp8, tag="f1", name=f"f1_{b + 1}")
                    state[b + 1] = (xtn, f1n)
                    # all DMAs go up front (ahead of this batch's stores in
                    # the SWDGE queue), but the q2/q3 ACT casts are emitted
                    # mid-loop: they would otherwise head-of-line-block the
                    # f1t copies that feed the early S(b+1) pairs
                    for q in (0, 1):
                        emit_loads_q(b + 1, f1n, xtn, q)
                    for q in (2, 3):
                        emit_dma_q(b + 1, xtn, q)
                    # interleave fc(b) m-groups with xpose + S of b+1
                    s_ps_n = {
                        m: ps_s.tile([P, F], f32, tag="s", name=f"s{b + 1}_{m}")
                        for m in range(CT)
                    }
                    pairs_n = {}
                    # pairs lag one m-group so fc(b) m=0 is not stuck behind
                    # xpose(b+1) (which waits on loads(b+1)) in PE program
                    # order
                    g = 0
                    for h in range(2):
                        for m in range(CT):
                            if g == CAST_Q2_G:
                                emit_casts_q(b + 1, f1n, xtn, 2)
                            elif g == CAST_Q3_G:
                                emit_casts_q(b + 1, f1n, xtn, 3)
                            for k in GRP_PAIRS[g]:
                                pairs_n[k] = emit_xpose_pair(b + 1, f1n, k)
                            if g == 7 and EARLY_STATS:
                                # last group: finish S(b+1) and emit its
                                # stats BEFORE this group's epilogues, so
                                # the reduce->exp->e2 chain overlaps the
                                # final epi drain instead of following it
                                for k in GRP_PAIRS[g]:
                                    emit_s_pair(b + 1, s_ps_n, pairs_n, k)
                                e2s = emit_stats(b + 1, s_ps_n)
                                emit_fc_m(b, h, m, f1, gst, xt, g)
                            else:
                                emit_fc_m(b, h, m, f1, gst, xt, g)
                                for k in GRP_PAIRS[g]:
                                    emit_s_pair(b + 1, s_ps_n, pairs_n, k)
                            g += 1
                    s_ps, pairs = s_ps_n, pairs_n
                else:
                    g = 0
                    for h in range(2):
                        for m in range(CT):
                            emit_fc_m(b, h, m, f1, gst, xt, g)
                            g += 1

    nc.finalize()
    return nc


def _get_nc():
    if "nc" not in _CACHE:
        _CACHE["nc"] = _build()
    return _CACHE["nc"]


def kernel(x: np.ndarray, beta: np.ndarray, **kw) -> np.ndarray:
    from concourse.bass_utils import run_bass_kernel_spmd

    x = np.ascontiguousarray(np.asarray(x, dtype=np.float32))
    beta = np.ascontiguousarray(np.asarray(beta, dtype=np.float32))
    assert x.shape == (B, C, 64, 64), x.shape

    xr = x.reshape(B, C, HW)
    in_maps = [
        {"x": np.ascontiguousarray(xr[i * BL : (i + 1) * BL]), "beta": beta}
        for i in range(NCORES)
    ]
    nc = _get_nc()
    res = run_bass_kernel_spmd(nc, in_maps, core_ids=list(range(NCORES)))
    out = np.concatenate(
        [np.asarray(r["y"], dtype=np.float32) for r in res.results], axis=0
    )
    return out.reshape(B, C, 64, 64)
